# revision 1
# baseline (speedup 1.0000x reference)
"""Trainium2 Bass kernel for the multi-task ActorNetwork (moe_routing).

Architecture (reference): per-sample expert routing over G=8 tasks:
    h1 = relu(x @ W1[idx] + b1[idx])     x:[B,376]  W1:[8,376,400]
    hf = relu(h1 @ W2 + b2)              W2:[400,300]
    a  = tanh(hf @ W3[idx] + b3[idx])    W3:[8,300,17]

Strategy: idx is sorted, and G == n_cores == 8, so we route on the HOST:
core g receives exactly the contiguous rows with idx == g (zero-padded to a
common BM), plus only ITS expert weights. Each core then runs a dense 3-layer
MLP -- no device-side routing, no collectives, and none of the 8x dense
compute the reference does.

Numerics: fp16 operands with fp32 PSUM accumulation (fp16 matmul streams at
1 cycle/row on the PE vs 4 for fp32; measured end-to-end max-abs error vs the
fp32 reference ~5e-3 on unit-scale outputs).

Layout: all matmuls keep the contraction dim on SBUF partitions:
    L1: h1T[h1, b] = relu(W1[d,h1].T @ xT[d,b] + b1)   (xT pre-transposed on host)
    L2: hfT[h2, b] = relu(W2[h1,h2].T @ h1T[h1,b] + b2)
    L3: aT[a, b]   = tanh(W3[h2,a].T @ hfT[h2,b] + b3) (host transposes back)
Biases ride the per-partition bias operand of the PSUM-eviction op (ACT
activation / DVE tensor_scalar), so each layer is matmul + one eviction op.

Engine split: PE matmuls; ACT does L1-relu + L3-tanh; DVE does L2-relu;
x-chunks stream on the SP HWDGE ring, weights on the Pool SWDGE ring, outputs
on the ACT HWDGE ring. A few dummy matmuls at t=0 warm the PE p-state while
the first DMAs land.
"""

import sys

if "/opt/trn_rl_repo" not in sys.path:
    sys.path.insert(0, "/opt/trn_rl_repo")

from contextlib import ExitStack

import numpy as np

import concourse.bass as bass
import concourse.mybir as mybir
from concourse.bass_utils import run_bass_kernel_spmd
from concourse.tile import TileContext

D, G, H1, H2, A = 376, 8, 400, 300, 17
P = 128
NCORES = 8
F16 = mybir.dt.float16
F32 = mybir.dt.float32


def _chunks(total, step):
    return [(o, min(step, total - o)) for o in range(0, total, step)]


K1 = _chunks(D, P)  # contraction tiles, layer 1: (128,128,120)
M1 = _chunks(H1, P)  # output-row tiles,  layer 1: (128,128,128,16)
K2 = M1  # contraction tiles, layer 2 == layer-1 output tiling
M2 = _chunks(H2, P)  # output-row tiles,  layer 2: (128,128,44)
K3 = M2  # contraction tiles, layer 3 == layer-2 output tiling

# K-tiles are packed along the free dim of one 128-partition tensor
# (zero-padded rows contribute nothing to the contraction), so each x chunk
# and each weight matrix moves in ONE DMA instead of one per K-tile
NK1, NK2, NK3 = len(K1), len(K2), len(K3)

# packed per-partition bias columns: [128, 8] = b1 x4 | b2 x3 | b3 x1
BIAS_COLS = len(M1) + len(M2) + 1

_nc_cache = {}
last_run = None  # BassKernelResults of the most recent launch (for profiling)
_last_in_maps = None  # per-core input dicts of the most recent launch

_nop_counter = [0]


def _legalize_wait_counts(nc):
    """This container's walrus encodes at most ONE sync-wait per instruction
    (DMA pseudo-instructions especially). Tile freely emits several. Sequencers
    are in-order, so hoisting the surplus waits onto same-engine NoOps placed
    immediately before the instruction is semantics-preserving."""
    for fn in nc.m.functions:
        for bb in fn.blocks:
            insts = list(bb.instructions)
            out = []
            changed = False
            for inst in insts:
                si = inst.sync_info
                waits = list(si.on_wait) if si is not None and si.on_wait else []
                if len(waits) > 1:
                    changed = True
                    for w in waits[:-1]:
                        _nop_counter[0] += 1
                        nop = mybir.InstNoOp(
                            name=f"waitsplit_nop_{_nop_counter[0]}",
                            engine=inst.engine,
                            ins=[],
                            outs=[],
                            sync_info=mybir.SyncInfo(on_wait=[w], on_update=[]),
                        )
                        out.append(nop)
                    si.on_wait = waits[-1:]
                out.append(inst)
            if changed:
                bb.instructions = out
    return nc


def _build(BM, legalize=True, reps=1):
    """Bass program for one core: dense 3-layer MLP over BM rows.

    reps>1 wraps the body in a hardware For_i loop (benchmarking only)."""
    bchunks = _chunks(BM, 512)

    nc = bass.Bass()
    xP = nc.declare_dram_parameter("xP", [P, NK1, BM], F16, isOutput=False)
    w1 = nc.declare_dram_parameter("w1", [P, NK1, H1], F16, isOutput=False)
    w2 = nc.declare_dram_parameter("w2", [P, NK2, H2], F16, isOutput=False)
    w3 = nc.declare_dram_parameter("w3", [P, NK3, A], F16, isOutput=False)
    bias = nc.declare_dram_parameter("bias", [P, BIAS_COLS], F32, isOutput=False)
    out = nc.declare_dram_parameter("out", [A, BM], F32, isOutput=True)

    Relu = mybir.ActivationFunctionType.Relu
    Tanh = mybir.ActivationFunctionType.Tanh
    Add = mybir.AluOpType.add
    Max = mybir.AluOpType.max

    with TileContext(nc) as tc, ExitStack() as ctx:
        wpool = ctx.enter_context(tc.tile_pool(name="w", bufs=1))
        xpool = ctx.enter_context(tc.tile_pool(name="x", bufs=3))
        h1pool = ctx.enter_context(tc.tile_pool(name="h1", bufs=3))
        hfpool = ctx.enter_context(tc.tile_pool(name="hf", bufs=3))
        opool = ctx.enter_context(tc.tile_pool(name="o", bufs=3))
        ps1 = ctx.enter_context(tc.tile_pool(name="ps1", bufs=5, space="PSUM"))
        ps2 = ctx.enter_context(tc.tile_pool(name="ps2", bufs=2, space="PSUM"))
        ps3 = ctx.enter_context(tc.tile_pool(name="ps3", bufs=1, space="PSUM"))

        def load_weights(param, nk, ncols, name, eng):
            # separate plain-2D tile per K-slab: keeps each lhsT slice a
            # maximally conventional AP so walrus's fast-weight-load (FWL)
            # detection is never defeated by 3D tile pitch
            tiles = []
            for ki in range(nk):
                t = wpool.tile([P, ncols], F16, tag=f"{name}_{ki}")
                eng.dma_start(out=t[:, :], in_=param[:, ki, :])
                tiles.append(t)
            return tiles

        w1_t = load_weights(w1, NK1, H1, "w1", nc.gpsimd)
        bias_t = wpool.tile([P, BIAS_COLS], F32, tag="bias")
        nc.gpsimd.dma_start(out=bias_t[:, :], in_=bias[:, :])
        w2_t = load_weights(w2, NK2, H2, "w2", nc.scalar)
        w3_t = load_weights(w3, NK3, A, "w3", nc.scalar)

        def b1_ap(mi, ms):
            return bias_t[:ms, mi : mi + 1]

        def b2_ap(mi, ms):
            return bias_t[:ms, len(M1) + mi : len(M1) + mi + 1]

        def b3_ap():
            return bias_t[:A, BIAS_COLS - 1 : BIAS_COLS]

        # p-state warmup: ~3us of dummy matmuls on zeroed SBUF while the
        # first DMAs are in flight, so the real matmuls run at 2.4 GHz
        warm = wpool.tile([P, P], F16, tag="warm")
        nc.vector.memset(warm[:, :], 0.0)
        for _ in range(16):
            pw = ps3.tile([P, 512], F32, tag="ps3")
            nc.tensor.matmul(
                pw[:, :P], warm[:, :P], warm[:, :P], start=True, stop=True
            )

        def emit_l1(b0, nb):
            # one packed DMA brings all NK1 K-tiles of this chunk
            xt = xpool.tile([P, NK1, 512], F16, tag="x")
            nc.sync.dma_start(out=xt[:, :, :nb], in_=xP[:, :, b0 : b0 + nb])

            # ---- layer 1: h1T[h1, b] = relu(W1.T @ xT + b1) ----
            h1_t = []
            for mi, (m0, ms) in enumerate(M1):
                pt = ps1.tile([P, 512], F32, tag="ps1")
                for ki in range(NK1):
                    nc.tensor.matmul(
                        pt[:ms, :nb],
                        w1_t[ki][:, m0 : m0 + ms],
                        xt[:, ki, :nb],
                        start=(ki == 0),
                        stop=(ki == NK1 - 1),
                    )
                ht = h1pool.tile([ms, nb], F16, tag=f"h1_{mi}")
                if mi == len(M1) - 1:
                    # the 16-row remainder costs a full tile-pass on whichever
                    # engine runs it; DVE has the most slack
                    nc.vector.tensor_scalar(
                        ht[:ms, :nb], pt[:ms, :nb], b1_ap(mi, ms), 0.0, op0=Add, op1=Max
                    )
                else:
                    nc.scalar.activation(
                        ht[:ms, :nb], pt[:ms, :nb], Relu, bias=b1_ap(mi, ms)
                    )
                h1_t.append(ht)
            return h1_t

        def emit_l2(h1_t, nb):
            # ---- layer 2: hfT[h2, b] = relu(W2.T @ h1T + b2), relu on DVE ----
            hf_t = []
            for mi, (m0, ms) in enumerate(M2):
                pt = ps2.tile([P, 512], F32, tag="ps2")
                for ki in range(NK2):
                    ks = K2[ki][1]
                    nc.tensor.matmul(
                        pt[:ms, :nb],
                        w2_t[ki][:ks, m0 : m0 + ms],
                        h1_t[ki][:, :nb],
                        start=(ki == 0),
                        stop=(ki == NK2 - 1),
                    )
                ht = hfpool.tile([ms, nb], F16, tag=f"hf_{mi}")
                nc.vector.tensor_scalar(
                    ht[:ms, :nb], pt[:ms, :nb], b2_ap(mi, ms), 0.0, op0=Add, op1=Max
                )
                hf_t.append(ht)
            return hf_t

        def emit_l3(hf_t, b0, nb):
            # ---- layer 3: aT[a, b] = tanh(W3.T @ hfT + b3) ----
            pt = ps3.tile([P, 512], F32, tag="ps3")
            for ki in range(NK3):
                ks = K3[ki][1]
                nc.tensor.matmul(
                    pt[:A, :nb],
                    w3_t[ki][:ks, :A],
                    hf_t[ki][:, :nb],
                    start=(ki == 0),
                    stop=(ki == NK3 - 1),
                )
            ot = opool.tile([A, nb], F32, tag="o")
            nc.scalar.activation(ot[:A, :nb], pt[:A, :nb], Tanh, bias=b3_ap())
            # out DMA on the ACT HWDGE ring: it trails tanh on the same
            # sequencer, so its wait never blocks the SP ring's x-prefetches
            nc.scalar.dma_start(out=out[:, b0 : b0 + nb], in_=ot[:A, :nb])

        def emit_all():
            # software-pipelined emission: L3 of chunk c-1 sits between L1(c)
            # and L2(c) in the PE stream, so the PE never waits on a relu that
            # was issued immediately before
            pending = None
            for b0, nb in bchunks:
                h1_t = emit_l1(b0, nb)
                if pending is not None:
                    emit_l3(*pending)
                hf_t = emit_l2(h1_t, nb)
                pending = (hf_t, b0, nb)
            emit_l3(*pending)

        if reps > 1:
            with tc.For_i(0, reps, 1):
                emit_all()
        else:
            emit_all()
    return _legalize_wait_counts(nc) if legalize else nc


def _get_nc(BM):
    if BM not in _nc_cache:
        _nc_cache[BM] = _build(BM)
    return _nc_cache[BM]


def pack_k(mat, nk):
    # [K, N] -> zero-pad K to nk*128 -> [128, nk, N] with row j*128+p of the
    # original at [p, j, :] (zero rows contribute nothing to the contraction)
    kk, nn = mat.shape
    pad = np.zeros((nk * P, nn), np.float16)
    pad[:kk] = mat.astype(np.float16)
    return np.ascontiguousarray(pad.reshape(nk, P, nn).transpose(1, 0, 2))


def pack_bias(b1g, b2s, b3g):
    pk = np.zeros((P, BIAS_COLS), np.float32)
    for mi, (m0, ms) in enumerate(M1):
        pk[:ms, mi] = b1g[m0 : m0 + ms]
    for mi, (m0, ms) in enumerate(M2):
        pk[:ms, len(M1) + mi] = b2s[m0 : m0 + ms]
    pk[:A, BIAS_COLS - 1] = b3g
    return pk


def kernel(state, idx, W1, b1, W2, b2, W3, b3):
    global last_run
    state = np.asarray(state, dtype=np.float32)
    idx = np.asarray(idx)
    W1 = np.asarray(W1, dtype=np.float32)
    b1 = np.asarray(b1, dtype=np.float32)
    W2 = np.asarray(W2, dtype=np.float32)
    b2 = np.asarray(b2, dtype=np.float32)
    W3 = np.asarray(W3, dtype=np.float32)
    b3 = np.asarray(b3, dtype=np.float32)
    B = state.shape[0]

    # Host-side routing: idx is sorted in the reference workload; fall back to
    # a stable argsort if not, so grouping stays correct for any input.
    idx_i = idx.astype(np.int64)
    perm = None
    if np.any(np.diff(idx_i) < 0):
        perm = np.argsort(idx_i, kind="stable")
        idx_i = idx_i[perm]
        state = state[perm]
    assert idx_i.min() >= 0 and idx_i.max() < G, "idx out of range [0, G)"
    counts = np.bincount(idx_i, minlength=G)[:G]
    offs = np.concatenate([[0], np.cumsum(counts)])

    BM = max(512, int(-(-counts.max() // P) * P))  # round up to 128 rows
    nc = _get_nc(BM)

    w2p = pack_k(W2, NK2)

    in_maps = []
    for g in range(G):
        seg = state[offs[g] : offs[g + 1]]
        xg = np.zeros((D, BM), np.float32)
        xg[:, : seg.shape[0]] = seg.T
        in_maps.append(
            {
                "xP": pack_k(xg, NK1),
                "w1": pack_k(W1[g], NK1),
                "w2": w2p,
                "w3": pack_k(W3[g], NK3),
                "bias": pack_bias(b1[g], b2, b3[g]),
            }
        )

    globals()["_last_in_maps"] = in_maps
    try:
        last_run = run_bass_kernel_spmd(nc, in_maps, list(range(NCORES)))
    except ModuleNotFoundError:
        # BASS_TRACE set in an env without the axon NTFF hook: retry untraced
        import os

        os.environ["BASS_NEVER_TRACE"] = "1"
        last_run = run_bass_kernel_spmd(nc, in_maps, list(range(NCORES)))

    out = np.empty((B, A), np.float32)
    for g in range(G):
        og = np.asarray(last_run.results[g]["out"])  # [A, BM]
        out[offs[g] : offs[g + 1]] = og.T[: counts[g]]
    if perm is not None:
        inv = np.empty_like(perm)
        inv[perm] = np.arange(B)
        out = out[inv]
    return out



# revision 4
# speedup vs baseline: 1.0523x; 1.0523x over previous
"""Trainium2 Bass kernel for the multi-task ActorNetwork (moe_routing).

Architecture (reference): per-sample expert routing over G=8 tasks:
    h1 = relu(x @ W1[idx] + b1[idx])     x:[B,376]  W1:[8,376,400]
    hf = relu(h1 @ W2 + b2)              W2:[400,300]
    a  = tanh(hf @ W3[idx] + b3[idx])    W3:[8,300,17]

Strategy: idx is sorted, and G == n_cores == 8, so we route on the HOST:
core g receives exactly the contiguous rows with idx == g (zero-padded to a
common BM), plus only ITS expert weights. Each core then runs a dense 3-layer
MLP -- no device-side routing, no collectives, and none of the 8x dense
compute the reference does.

Numerics: fp16 operands with fp32 PSUM accumulation (fp16 matmul streams at
1 cycle/row on the PE vs 4 for fp32; measured end-to-end max-abs error vs the
fp32 reference ~5e-3 on unit-scale outputs).

Layout: layers 1-2 keep the contraction dim on SBUF partitions and stream the
batch as the matmul moving dim (feature-major):
    L1: h1T[h1, b] = relu(W1[d,h1].T @ xT[d,b] + b1)   (xT pre-transposed on host)
    L2: hfT[h2, b] = relu(W2[h1,h2].T @ h1T[h1,b] + b2)
Layer 3 is batch-major: the PE streams only the 17 output features per
128-sample slice (lhsT = a 128-column slice of hfT, rhs = W3), so L3 costs
~17 cycles per slice-pass instead of 512 per K-pass:
    L3: a[b_slice, a17] += hfT[h2, b_slice].T @ W3[h2, a17]
b3 rides a rank-1 matmul (ones[1,b_slice].T @ b3[1,17]) into the same PSUM
accumulation group; all L3 matmuls of a chunk share one PSUM bank / one
start..stop group writing disjoint 17-column slices.

Both L1 and L2 emit K-outer round-robin (M0K0,M1K0,..,M0K1,..) so each h1/hf
tile eviction lands well before the pass that consumes it -- the PE never
waits on ACT/DVE.

Engine split: PE matmuls; ACT does L1-relu (3 tiles) + L3-tanh; DVE does the
L1 remainder + L2-relu; x-chunks stream on the SP HWDGE ring, weights on the
Pool SWDGE ring, outputs on the ACT HWDGE ring.
"""

import sys

if "/opt/trn_rl_repo" not in sys.path:
    sys.path.insert(0, "/opt/trn_rl_repo")

from contextlib import ExitStack

import numpy as np

import concourse.bass as bass
import concourse.mybir as mybir
from concourse.bass_utils import run_bass_kernel_spmd
from concourse.tile import TileContext

D, G, H1, H2, A = 376, 8, 400, 300, 17
P = 128
NCORES = 8
F16 = mybir.dt.float16
F32 = mybir.dt.float32


def _chunks(total, step):
    return [(o, min(step, total - o)) for o in range(0, total, step)]


K1 = _chunks(D, P)  # contraction tiles, layer 1: (128,128,120)
M1 = _chunks(H1, P)  # output-row tiles,  layer 1: (128,128,128,16)
K2 = M1  # contraction tiles, layer 2 == layer-1 output tiling
M2 = _chunks(H2, P)  # output-row tiles,  layer 2: (128,128,44)
K3 = M2  # contraction tiles, layer 3 == layer-2 output tiling

# K-tiles are packed along the free dim of one 128-partition tensor
# (zero-padded rows contribute nothing to the contraction), so each x chunk
# and each weight matrix moves in ONE DMA instead of one per K-tile
NK1, NK2, NK3 = len(K1), len(K2), len(K3)

# packed per-partition bias columns: [128, 7] = b1 x4 | b2 x3
BIAS_COLS = len(M1) + len(M2)

_nc_cache = {}
last_run = None  # BassKernelResults of the most recent launch (for profiling)
_last_in_maps = None  # per-core input dicts of the most recent launch

_nop_counter = [0]


def _legalize_wait_counts(nc):
    """This container's walrus encodes at most ONE sync-wait per instruction
    (DMA pseudo-instructions especially). Tile freely emits several. Sequencers
    are in-order, so hoisting the surplus waits onto same-engine NoOps placed
    immediately before the instruction is semantics-preserving."""
    for fn in nc.m.functions:
        for bb in fn.blocks:
            insts = list(bb.instructions)
            out = []
            changed = False
            for inst in insts:
                si = inst.sync_info
                waits = list(si.on_wait) if si is not None and si.on_wait else []
                if len(waits) > 1:
                    changed = True
                    for w in waits[:-1]:
                        _nop_counter[0] += 1
                        nop = mybir.InstNoOp(
                            name=f"waitsplit_nop_{_nop_counter[0]}",
                            engine=inst.engine,
                            ins=[],
                            outs=[],
                            sync_info=mybir.SyncInfo(on_wait=[w], on_update=[]),
                        )
                        out.append(nop)
                    si.on_wait = waits[-1:]
                out.append(inst)
            if changed:
                bb.instructions = out
    return nc


def _build(BM, legalize=True, reps=1):
    """Bass program for one core: dense 3-layer MLP over BM rows.

    reps>1 wraps the body in a hardware For_i loop (benchmarking only)."""
    bchunks = _chunks(BM, 512)
    NS = BM // P  # 128-row output slices

    nc = bass.Bass()
    xP = nc.declare_dram_parameter("xP", [P, NK1, BM], F16, isOutput=False)
    w1 = nc.declare_dram_parameter("w1", [P, NK1, H1], F16, isOutput=False)
    w2 = nc.declare_dram_parameter("w2", [P, NK2, H2], F16, isOutput=False)
    w3 = nc.declare_dram_parameter("w3", [P, NK3, A], F16, isOutput=False)
    b3r = nc.declare_dram_parameter("b3r", [1, A], F16, isOutput=False)
    bias = nc.declare_dram_parameter("bias", [P, BIAS_COLS], F32, isOutput=False)
    # out[p, s, a] = action a of sample s*128 + p (host re-interleaves)
    out = nc.declare_dram_parameter("out", [P, NS, A], F32, isOutput=True)

    Relu = mybir.ActivationFunctionType.Relu
    Tanh = mybir.ActivationFunctionType.Tanh
    Add = mybir.AluOpType.add
    Max = mybir.AluOpType.max

    with TileContext(nc) as tc, ExitStack() as ctx:
        wpool = ctx.enter_context(tc.tile_pool(name="w", bufs=1))
        xpool = ctx.enter_context(tc.tile_pool(name="x", bufs=3))
        h1pool = ctx.enter_context(tc.tile_pool(name="h1", bufs=3))
        hfpool = ctx.enter_context(tc.tile_pool(name="hf", bufs=3))
        opool = ctx.enter_context(tc.tile_pool(name="o", bufs=3))
        ps1 = ctx.enter_context(tc.tile_pool(name="ps1", bufs=4, space="PSUM"))
        ps2 = ctx.enter_context(tc.tile_pool(name="ps2", bufs=3, space="PSUM"))
        ps3 = ctx.enter_context(tc.tile_pool(name="ps3", bufs=1, space="PSUM"))

        def load_weights(param, nk, ncols, name, eng):
            # separate plain-2D tile per K-slab: keeps each lhsT slice a
            # maximally conventional AP so walrus's fast-weight-load (FWL)
            # detection is never defeated by 3D tile pitch
            tiles = []
            for ki in range(nk):
                t = wpool.tile([P, ncols], F16, tag=f"{name}_{ki}")
                eng.dma_start(out=t[:, :], in_=param[:, ki, :])
                tiles.append(t)
            return tiles

        w1_t = load_weights(w1, NK1, H1, "w1", nc.gpsimd)
        bias_t = wpool.tile([P, BIAS_COLS], F32, tag="bias")
        nc.gpsimd.dma_start(out=bias_t[:, :], in_=bias[:, :])
        w3_t = load_weights(w3, NK3, A, "w3", nc.gpsimd)
        b3r_t = wpool.tile([1, A], F16, tag="b3r")
        nc.gpsimd.dma_start(out=b3r_t[:, :], in_=b3r[:, :])
        w2_t = load_weights(w2, NK2, H2, "w2", nc.scalar)
        # all-ones row for the rank-1 bias matmul of L3
        ones_t = wpool.tile([1, 512], F16, tag="ones")
        nc.vector.memset(ones_t[:, :], 1.0)

        def b1_ap(mi, ms):
            return bias_t[:ms, mi : mi + 1]

        def b2_ap(mi, ms):
            return bias_t[:ms, len(M1) + mi : len(M1) + mi + 1]

        def emit_l1(ci, b0, nb):
            # one packed DMA brings all NK1 K-tiles of this chunk (chunk 0:
            # one DMA per K-slab so the first matmul isn't gated on the full
            # chunk transfer)
            xt = xpool.tile([P, NK1, 512], F16, tag="x")
            if ci == 0:
                for ki in range(NK1):
                    nc.sync.dma_start(
                        out=xt[:, ki, :nb], in_=xP[:, ki, b0 : b0 + nb]
                    )
            else:
                nc.sync.dma_start(out=xt[:, :, :nb], in_=xP[:, :, b0 : b0 + nb])

            # ---- layer 1: h1T[h1, b] = relu(W1.T @ xT + b1) ----
            # K-outer round-robin; eviction follows each M-tile's last K pass
            pts = [ps1.tile([P, 512], F32, tag="ps1", name=f"ps1_{ci}_{i}") for i in range(len(M1))]
            h1_t = [None] * len(M1)
            for ki in range(NK1):
                for mi, (m0, ms) in enumerate(M1):
                    nc.tensor.matmul(
                        pts[mi][:ms, :nb],
                        w1_t[ki][:, m0 : m0 + ms],
                        xt[:, ki, :nb],
                        start=(ki == 0),
                        stop=(ki == NK1 - 1),
                    )
                    if ki == NK1 - 1:
                        ht = h1pool.tile([ms, nb], F16, tag=f"h1_{mi}")
                        if mi == len(M1) - 1:
                            # the 16-row remainder goes to DVE; ACT has the
                            # L3 tanh + out DMA as well
                            nc.vector.tensor_scalar(
                                ht[:ms, :nb],
                                pts[mi][:ms, :nb],
                                b1_ap(mi, ms),
                                0.0,
                                op0=Add,
                                op1=Max,
                            )
                        else:
                            nc.scalar.activation(
                                ht[:ms, :nb], pts[mi][:ms, :nb], Relu, bias=b1_ap(mi, ms)
                            )
                        h1_t[mi] = ht
            return h1_t

        def emit_l2(h1_t, nb):
            # ---- layer 2: hfT[h2, b] = relu(W2.T @ h1T + b2), relu on DVE ----
            # K-outer round-robin: the K3 (16-row h1 remainder) passes come
            # last, giving the DVE eviction of h1_3 a full round to land
            pts = [ps2.tile([P, 512], F32, tag="ps2", name=f"ps2_{i}") for i in range(len(M2))]
            hf_t = [None] * len(M2)
            for ki in range(NK2):
                ks = K2[ki][1]
                for mi, (m0, ms) in enumerate(M2):
                    nc.tensor.matmul(
                        pts[mi][:ms, :nb],
                        w2_t[ki][:ks, m0 : m0 + ms],
                        h1_t[ki][:, :nb],
                        start=(ki == 0),
                        stop=(ki == NK2 - 1),
                    )
                    if ki == NK2 - 1:
                        ht = hfpool.tile([ms, nb], F16, tag=f"hf_{mi}")
                        nc.vector.tensor_scalar(
                            ht[:ms, :nb],
                            pts[mi][:ms, :nb],
                            b2_ap(mi, ms),
                            0.0,
                            op0=Add,
                            op1=Max,
                        )
                        hf_t[mi] = ht
            return hf_t

        def emit_l3(hf_t, b0, nb, ci):
            # ---- layer 3 (batch-major): a[b_slice, :] = tanh(hfT_slice.T @ W3
            #      + ones.T @ b3) ----
            # All matmuls of the chunk form ONE PSUM accumulation group in one
            # bank, writing disjoint 17-column slices; each slice's first
            # matmul lands on pending-zero bytes (= start semantics for that
            # slice), the rest accumulate.
            ns = (nb + P - 1) // P
            pt = ps3.tile([P, 512], F32, tag="ps3")
            n_mm = ns * (NK3 + 1)
            i = 0
            for s in range(ns):
                c0, c1 = s * P, min((s + 1) * P, nb)
                o = pt[:, s * A : s * A + A][: c1 - c0, :]
                for ki in range(NK3):
                    ks = K3[ki][1]
                    nc.tensor.matmul(
                        o,
                        hf_t[ki][:, c0:c1],
                        w3_t[ki][:ks, :A],
                        start=(i == 0),
                        stop=(i == n_mm - 1),
                        skip_group_check=True,
                    )
                    i += 1
                # rank-1 bias: ones[1, c0:c1].T @ b3[1, :A]
                nc.tensor.matmul(
                    o,
                    ones_t[:, c0:c1],
                    b3r_t[:, :A],
                    start=False,
                    stop=(i == n_mm - 1),
                    skip_group_check=True,
                )
                i += 1
            # single tanh eviction for the whole chunk, then out DMA on the
            # ACT HWDGE ring (trails tanh on the same sequencer)
            ot = opool.tile([P, 4, A], F32, tag="o")
            nc.scalar.activation(ot[:, :ns, :], pt[:, : ns * A], Tanh)
            s0 = b0 // P
            nc.scalar.dma_start(out=out[:, s0 : s0 + ns, :], in_=ot[:, :ns, :])

        def emit_all():
            # software-pipelined emission: L3 of chunk c-1 sits between L1(c)
            # and L2(c) in the PE stream, covering the hf-eviction latency
            pending = None
            for ci, (b0, nb) in enumerate(bchunks):
                h1_t = emit_l1(ci, b0, nb)
                if pending is not None:
                    emit_l3(*pending)
                hf_t = emit_l2(h1_t, nb)
                pending = (hf_t, b0, nb, ci)
            emit_l3(*pending)

        if reps > 1:
            with tc.For_i(0, reps, 1):
                emit_all()
        else:
            emit_all()
    return _legalize_wait_counts(nc) if legalize else nc


def _get_nc(BM):
    if BM not in _nc_cache:
        _nc_cache[BM] = _build(BM)
    return _nc_cache[BM]


def pack_k(mat, nk):
    # [K, N] -> zero-pad K to nk*128 -> [128, nk, N] with row j*128+p of the
    # original at [p, j, :] (zero rows contribute nothing to the contraction)
    kk, nn = mat.shape
    pad = np.zeros((nk * P, nn), np.float16)
    pad[:kk] = mat.astype(np.float16)
    return np.ascontiguousarray(pad.reshape(nk, P, nn).transpose(1, 0, 2))


def pack_bias(b1g, b2s):
    pk = np.zeros((P, BIAS_COLS), np.float32)
    for mi, (m0, ms) in enumerate(M1):
        pk[:ms, mi] = b1g[m0 : m0 + ms]
    for mi, (m0, ms) in enumerate(M2):
        pk[:ms, len(M1) + mi] = b2s[m0 : m0 + ms]
    return pk


def kernel(state, idx, W1, b1, W2, b2, W3, b3):
    global last_run
    state = np.asarray(state, dtype=np.float32)
    idx = np.asarray(idx)
    W1 = np.asarray(W1, dtype=np.float32)
    b1 = np.asarray(b1, dtype=np.float32)
    W2 = np.asarray(W2, dtype=np.float32)
    b2 = np.asarray(b2, dtype=np.float32)
    W3 = np.asarray(W3, dtype=np.float32)
    b3 = np.asarray(b3, dtype=np.float32)
    B = state.shape[0]

    # Host-side routing: idx is sorted in the reference workload; fall back to
    # a stable argsort if not, so grouping stays correct for any input.
    idx_i = idx.astype(np.int64)
    perm = None
    if np.any(np.diff(idx_i) < 0):
        perm = np.argsort(idx_i, kind="stable")
        idx_i = idx_i[perm]
        state = state[perm]
    assert idx_i.min() >= 0 and idx_i.max() < G, "idx out of range [0, G)"
    counts = np.bincount(idx_i, minlength=G)[:G]
    offs = np.concatenate([[0], np.cumsum(counts)])

    BM = max(512, int(-(-counts.max() // P) * P))  # round up to 128 rows
    nc = _get_nc(BM)
    NS = BM // P

    w2p = pack_k(W2, NK2)

    in_maps = []
    for g in range(G):
        seg = state[offs[g] : offs[g + 1]]
        xg = np.zeros((D, BM), np.float32)
        xg[:, : seg.shape[0]] = seg.T
        in_maps.append(
            {
                "xP": pack_k(xg, NK1),
                "w1": pack_k(W1[g], NK1),
                "w2": w2p,
                "w3": pack_k(W3[g], NK3),
                "b3r": b3[g].astype(np.float16).reshape(1, A),
                "bias": pack_bias(b1[g], b2),
            }
        )

    globals()["_last_in_maps"] = in_maps
    try:
        last_run = run_bass_kernel_spmd(nc, in_maps, list(range(NCORES)))
    except ModuleNotFoundError:
        # BASS_TRACE set in an env without the axon NTFF hook: retry untraced
        import os

        os.environ["BASS_NEVER_TRACE"] = "1"
        last_run = run_bass_kernel_spmd(nc, in_maps, list(range(NCORES)))

    out = np.empty((B, A), np.float32)
    for g in range(G):
        og = np.asarray(last_run.results[g]["out"])  # [P, NS, A]
        rows = og.transpose(1, 0, 2).reshape(NS * P, A)
        out[offs[g] : offs[g + 1]] = rows[: counts[g]]
    if perm is not None:
        inv = np.empty_like(perm)
        inv[perm] = np.arange(B)
        out = out[inv]
    return out


# revision 8
# speedup vs baseline: 1.1687x; 1.1106x over previous
"""Trainium2 Bass kernel for the multi-task ActorNetwork (moe_routing).

Architecture (reference): per-sample expert routing over G=8 tasks:
    h1 = relu(x @ W1[idx] + b1[idx])     x:[B,376]  W1:[8,376,400]
    hf = relu(h1 @ W2 + b2)              W2:[400,300]
    a  = tanh(hf @ W3[idx] + b3[idx])    W3:[8,300,17]

Strategy: idx is sorted, and G == n_cores == 8, so we route on the HOST:
core g receives exactly the contiguous rows with idx == g (zero-padded to a
common BM), plus only ITS expert weights. Each core then runs a dense 3-layer
MLP -- no device-side routing, no collectives, and none of the 8x dense
compute the reference does.

Numerics: fp16 operands with fp32 PSUM accumulation; measured end-to-end
max-abs error vs the fp32 reference ~5e-3 on unit-scale outputs.

Matmul cost on the PE is (output free size) x (cycles/row), so the layout is
chosen to minimize streamed output elements:
  * L1/L2 main tiles (128-feature groups) run feature-major: the contraction
    dim sits on SBUF partitions and the 512-sample chunk streams as the
    moving dim.
  * The ragged remainders (h1's last 16 features, hf's last 44) run
    BATCH-major: out[b_slice, F] = x_sliceT.T @ W[:, rem] streams only F
    elements per pass, then a PE transpose (through fp16 PSUM, evicted by the
    otherwise-idle GPSIMD/Pool engine) restores the feature-major layout the
    next layer needs.  16 /44-feature remainders cost ~0.77k/1.4k cycles per
    chunk instead of 1.5k/2k.
  * L3 is fully batch-major (17 outputs): lhsT = a 128-column slice of hfT,
    rhs = W3 -- ~17 cycles per slice-pass instead of 512 per K-pass.  All L3
    matmuls of a chunk form ONE PSUM accumulation group in one bank writing
    disjoint 17-column slices.
Biases of batch-major outputs ride rank-1 matmuls (ones[1,b].T @ b_row[1,F])
inside the same accumulation group; feature-major evictions use the
per-partition bias operand.

The final (partial) chunk uses the plain feature-major path with M-outer
emission: it keeps the drain chain short and avoids PSUM WAR deadlocks with
3 live L2 groups on 2 banks.

Engine split: PE matmuls+transposes; ACT: L1 relu x2, BM16 relu, L3 tanh, out
DMA; DVE: L1 relu x1, L2 relu x2, BM44 relu; Pool: weight DMAs + transpose
evictions; SP: x-chunk streaming.  A dummy activation at t~0.5us preloads the
ACT function table off the critical path.
"""

import sys

if "/opt/trn_rl_repo" not in sys.path:
    sys.path.insert(0, "/opt/trn_rl_repo")

from contextlib import ExitStack

import numpy as np

import concourse.bass as bass
import concourse.mybir as mybir
from concourse.bass_utils import run_bass_kernel_spmd
from concourse.tile import TileContext

D, G, H1, H2, A = 376, 8, 400, 300, 17
P = 128
NCORES = 8
F16 = mybir.dt.float16
F32 = mybir.dt.float32

F1R = H1 - 3 * P  # 16: L1 feature remainder
F2R = H2 - 2 * P  # 44: L2 feature remainder


def _chunks(total, step):
    return [(o, min(step, total - o)) for o in range(0, total, step)]


K1 = _chunks(D, P)  # contraction tiles, layer 1: (128,128,120)
M1 = _chunks(H1, P)  # output-row tiles,  layer 1: (128,128,128,16)
K2 = M1  # contraction tiles, layer 2 == layer-1 output tiling
M2 = _chunks(H2, P)  # output-row tiles,  layer 2: (128,128,44)
K3 = M2  # contraction tiles, layer 3 == layer-2 output tiling

NK1, NK2, NK3 = len(K1), len(K2), len(K3)

# packed per-partition bias columns: [128, 7] = b1 x4 | b2 x3
BIAS_COLS = len(M1) + len(M2)

_nc_cache = {}
last_run = None  # BassKernelResults of the most recent launch (for profiling)
_last_in_maps = None  # per-core input dicts of the most recent launch

_nop_counter = [0]


def _legalize_wait_counts(nc):
    """This container's walrus encodes at most ONE sync-wait per instruction
    (DMA pseudo-instructions especially). Tile freely emits several. Sequencers
    are in-order, so hoisting the surplus waits onto same-engine NoOps placed
    immediately before the instruction is semantics-preserving."""
    for fn in nc.m.functions:
        for bb in fn.blocks:
            insts = list(bb.instructions)
            out = []
            changed = False
            for inst in insts:
                si = inst.sync_info
                waits = list(si.on_wait) if si is not None and si.on_wait else []
                if len(waits) > 1:
                    changed = True
                    for w in waits[:-1]:
                        _nop_counter[0] += 1
                        nop = mybir.InstNoOp(
                            name=f"waitsplit_nop_{_nop_counter[0]}",
                            engine=inst.engine,
                            ins=[],
                            outs=[],
                            sync_info=mybir.SyncInfo(on_wait=[w], on_update=[]),
                        )
                        out.append(nop)
                    si.on_wait = waits[-1:]
                out.append(inst)
            if changed:
                bb.instructions = out
    return nc


def _build(BM, legalize=True, reps=1):
    """Bass program for one core: dense 3-layer MLP over BM rows.

    reps>1 wraps the body in a hardware For_i loop (benchmarking only)."""
    bchunks = _chunks(BM, 512)
    NS = BM // P  # 128-row output slices

    nc = bass.Bass()
    xP = nc.declare_dram_parameter("xP", [P, NK1, BM], F16, isOutput=False)
    w1 = nc.declare_dram_parameter("w1", [P, NK1, H1], F16, isOutput=False)
    w2 = nc.declare_dram_parameter("w2", [P, NK2, H2], F16, isOutput=False)
    w3 = nc.declare_dram_parameter("w3", [P, NK3, A], F16, isOutput=False)
    # rank-1 bias rows: b1[384:400] | b2[256:300] | b3  (fp16)
    brow = nc.declare_dram_parameter("brow", [1, F1R + F2R + A], F16, isOutput=False)
    ident = nc.declare_dram_parameter("ident", [P, P], F16, isOutput=False)
    bias = nc.declare_dram_parameter("bias", [P, BIAS_COLS], F32, isOutput=False)
    # out[p, s, a] = action a of sample s*128 + p (host re-interleaves)
    out = nc.declare_dram_parameter("out", [P, NS, A], F32, isOutput=True)

    Relu = mybir.ActivationFunctionType.Relu
    Tanh = mybir.ActivationFunctionType.Tanh
    Add = mybir.AluOpType.add
    Max = mybir.AluOpType.max

    with TileContext(nc) as tc, ExitStack() as ctx:
        wpool = ctx.enter_context(tc.tile_pool(name="w", bufs=1))
        xpool = ctx.enter_context(tc.tile_pool(name="x", bufs=3))
        h1pool = ctx.enter_context(tc.tile_pool(name="h1", bufs=3))
        hfpool = ctx.enter_context(tc.tile_pool(name="hf", bufs=3))
        bmpool = ctx.enter_context(tc.tile_pool(name="bm", bufs=2))
        opool = ctx.enter_context(tc.tile_pool(name="o", bufs=3))
        ps1 = ctx.enter_context(tc.tile_pool(name="ps1", bufs=3, space="PSUM"))
        ps2 = ctx.enter_context(tc.tile_pool(name="ps2", bufs=2, space="PSUM"))
        psbm = ctx.enter_context(tc.tile_pool(name="psbm", bufs=1, space="PSUM"))
        pst = ctx.enter_context(tc.tile_pool(name="pst", bufs=1, space="PSUM"))
        ps3 = ctx.enter_context(tc.tile_pool(name="ps3", bufs=1, space="PSUM"))

        def load_weights(param, nk, ncols, name, eng):
            tiles = []
            for ki in range(nk):
                t = wpool.tile([P, ncols], F16, tag=f"{name}_{ki}")
                eng.dma_start(out=t[:, :], in_=param[:, ki, :])
                tiles.append(t)
            return tiles

        # Pool ring order is latency-tuned: w1 gates the first L1 passes,
        # brow/ident/bias gate mid-chunk-0 consumers, w3 is needed a full
        # chunk later.
        w1_t = load_weights(w1, NK1, H1, "w1", nc.gpsimd)
        brow_t = wpool.tile([1, F1R + F2R + A], F16, tag="brow")
        nc.gpsimd.dma_start(out=brow_t[:, :], in_=brow[:, :])
        ident_t = wpool.tile([P, P], F16, tag="ident")
        nc.gpsimd.dma_start(out=ident_t[:, :], in_=ident[:, :])
        bias_t = wpool.tile([P, BIAS_COLS], F32, tag="bias")
        nc.gpsimd.dma_start(out=bias_t[:, :], in_=bias[:, :])
        w3_t = load_weights(w3, NK3, A, "w3", nc.gpsimd)

        # all-ones row for the rank-1 bias matmuls
        ones_t = wpool.tile([1, 512], F16, tag="ones")
        nc.vector.memset(ones_t[:, :], 1.0)
        # preload the ACT function table off the critical path (the first
        # activation otherwise pays ~1.4us mid-stream)
        actw_t = wpool.tile([1, 1], F32, tag="actw")
        nc.scalar.activation(actw_t[:, :], ones_t[0:1, 0:1], Relu)

        w2_t = load_weights(w2, NK2, H2, "w2", nc.scalar)

        def b1_ap(mi, ms):
            return bias_t[:ms, mi : mi + 1]

        def b2_ap(mi, ms):
            return bias_t[:ms, len(M1) + mi : len(M1) + mi + 1]

        def emit_chunk(ci, b0, nb, prev):
            last = ci == len(bchunks) - 1
            bm = nb == 512 and not last
            ns = (nb + P - 1) // P

            # ---- x DMA (chunk 0: per-K-slab so the first passes start early)
            xt = xpool.tile([P, NK1, 512], F16, tag="x")
            if ci == 0:
                for ki in range(NK1):
                    nc.sync.dma_start(out=xt[:, ki, :nb], in_=xP[:, ki, b0 : b0 + nb])
            else:
                nc.sync.dma_start(out=xt[:, :, :nb], in_=xP[:, :, b0 : b0 + nb])

            # ---- P1: L1 feature-major tiles ----
            nfm1 = 3 if bm else len(M1)
            m1fm = M1[:nfm1]
            pts1 = [
                ps1.tile([P, 512], F32, tag="ps1", name=f"ps1_{ci}_{i}")
                for i in range(min(nfm1, 3))
            ]
            if not bm and nfm1 == 4:
                # last/partial chunk: M-outer, M3 reuses M0's bank after its
                # eviction (sequential groups -- no WAR deadlock)
                pts1.append(ps1.tile([P, 512], F32, tag="ps1", name=f"ps1_{ci}_3"))
            h1_t = [None] * NK2
            if ci == 0 and bm:
                order1 = [(ki, mi) for ki in range(NK1) for mi in range(nfm1)]
            else:
                order1 = [(ki, mi) for mi in range(nfm1) for ki in range(NK1)]
            for ki, mi in order1:
                m0, ms = M1[mi]
                nc.tensor.matmul(
                    pts1[mi][:ms, :nb],
                    w1_t[ki][:, m0 : m0 + ms],
                    xt[:, ki, :nb],
                    start=(ki == 0),
                    stop=(ki == NK1 - 1),
                )
                if ki == NK1 - 1:
                    ht = h1pool.tile([ms, nb], F16, tag=f"h1_{mi}")
                    if mi < 2:
                        nc.scalar.activation(
                            ht[:ms, :nb], pts1[mi][:ms, :nb], Relu, bias=b1_ap(mi, ms)
                        )
                    else:
                        nc.vector.tensor_scalar(
                            ht[:ms, :nb],
                            pts1[mi][:ms, :nb],
                            b1_ap(mi, ms),
                            0.0,
                            op0=Add,
                            op1=Max,
                        )
                    h1_t[mi] = ht

            # ---- P2: L1 batch-major remainder (16 features) ----
            ptbm = None
            bm1_sb = None
            if bm:
                # psbm bank: cols [0,4*F1R) = BM16, [4*F1R, 4*F1R+4*F2R) = BM44;
                # ONE accumulation group from the first BM16 mm to the last
                # BM44 bias mm (each slice's first write lands on pending-zero
                # bytes = per-slice start semantics).
                ptbm = psbm.tile([P, 4 * (F1R + F2R)], F32, tag="psbm")
                for s in range(4):
                    o = ptbm[:, s * F1R : (s + 1) * F1R]
                    c0 = s * P
                    for ki in range(NK1):
                        nc.tensor.matmul(
                            o,
                            xt[:, ki, c0 : c0 + P],
                            w1_t[ki][:, 3 * P : H1],
                            start=(s == 0 and ki == 0),
                            stop=False,
                            skip_group_check=True,
                        )
                    nc.tensor.matmul(
                        o,
                        ones_t[:, c0 : c0 + P],
                        brow_t[:, :F1R],
                        start=False,
                        stop=False,
                        skip_group_check=True,
                    )
                bm1_sb = bmpool.tile([P, 4, F1R], F16, tag="bm1")
                nc.scalar.activation(bm1_sb[:, :, :], ptbm[:, : 4 * F1R], Relu)

            # ---- P3: transpose the PREVIOUS chunk's BM44 back to
            #      feature-major (44 x 512) ----
            if prev is not None and prev["ptt"] is not None:
                ptt = prev["ptt"]
                for s in range(4):
                    nc.tensor.transpose(
                        ptt[:F2R, 512 + s * P : 512 + (s + 1) * P],
                        prev["bm2_sb"][:, s, :],
                        ident_t[:, :],
                    )
                # ACT as copy engine: values are post-relu (>=0) so Relu
                # is the identity; GPSIMD cannot touch PSUM per the verifier
                nc.scalar.activation(
                    prev["hf_t"][2][:F2R, :], ptt[:F2R, 512:1024], Relu
                )

            # ---- P4..P7: L2 feature-major K-rounds 0..2 (+T16 after K1) ----
            nfm2 = 2 if bm else len(M2)
            m2fm = M2[:nfm2]
            pts2 = [
                ps2.tile([P, 512], F32, tag="ps2", name=f"ps2_{ci}_{i}")
                for i in range(min(nfm2, 2))
            ]
            ptt = None
            h1r_sb = None
            if bm:
                ptt = pst.tile([P, 1024], F16, tag="pst")

            def l2_round(ki):
                ks = K2[ki][1]
                for mi, (m0, ms) in enumerate(m2fm):
                    nc.tensor.matmul(
                        pts2[mi][:ms, :nb],
                        w2_t[ki][:ks, m0 : m0 + ms],
                        h1_t[ki][:, :nb],
                        start=(ki == 0),
                        stop=(ki == NK2 - 1),
                    )

            if bm:
                l2_round(0)
                l2_round(1)
                # T16: transpose BM16 -> h1 remainder [16, 512]
                for s in range(4):
                    nc.tensor.transpose(
                        ptt[:F1R, s * P : (s + 1) * P],
                        bm1_sb[:, s, :],
                        ident_t[:, :],
                    )
                h1r_sb = h1pool.tile([F1R, 512], F16, tag="h1r")
                nc.vector.tensor_scalar(
                    h1r_sb[:, :], ptt[:F1R, :512], 0.0, None, op0=Add
                )
                h1_t[3] = h1r_sb
                l2_round(2)
            else:
                # M-outer for the drain chunk (see docstring)
                for mi, (m0, ms) in enumerate(m2fm):
                    for ki in range(NK2):
                        ks = K2[ki][1]
                        nc.tensor.matmul(
                            pts2[mi if mi < 2 else 0][:ms, :nb],
                            w2_t[ki][:ks, m0 : m0 + ms],
                            h1_t[ki][:, :nb],
                            start=(ki == 0),
                            stop=(ki == NK2 - 1),
                        )
                    ht = hfpool.tile([ms, nb], F16, tag=f"hf_{mi}")
                    if mi == 1:
                        nc.scalar.activation(
                            ht[:ms, :nb],
                            pts2[1][:ms, :nb],
                            Relu,
                            bias=b2_ap(mi, ms),
                        )
                    else:
                        nc.vector.tensor_scalar(
                            ht[:ms, :nb],
                            pts2[mi if mi < 2 else 0][:ms, :nb],
                            b2_ap(mi, ms),
                            0.0,
                            op0=Add,
                            op1=Max,
                        )
                    if mi == 0:
                        hf_t = [ht]
                    else:
                        hf_t.append(ht)

            # ---- P8: BM44 K0..K2 ----
            bm2_sb = None
            if bm:
                for s in range(4):
                    o = ptbm[:, 4 * F1R + s * F2R : 4 * F1R + (s + 1) * F2R]
                    c0 = s * P
                    for ki in range(3):
                        ks = K2[ki][1]
                        nc.tensor.matmul(
                            o,
                            h1_t[ki][:ks, c0 : c0 + P],
                            w2_t[ki][:ks, 2 * P : H2],
                            start=False,
                            stop=False,
                            skip_group_check=True,
                        )

            # ---- P9: L3 of the previous chunk, tanh, out DMA ----
            if prev is not None:
                emit_l3(prev)

            if bm:
                # ---- P10: L2 K3 round (16-row h1 remainder) + hf evictions
                hf_t = []
                ki = NK2 - 1
                ks = K2[ki][1]
                for mi, (m0, ms) in enumerate(m2fm):
                    nc.tensor.matmul(
                        pts2[mi][:ms, :nb],
                        w2_t[ki][:ks, m0 : m0 + ms],
                        h1_t[ki][:, :nb],
                        start=False,
                        stop=True,
                    )
                    ht = hfpool.tile([ms, nb], F16, tag=f"hf_{mi}")
                    nc.vector.tensor_scalar(
                        ht[:ms, :nb],
                        pts2[mi][:ms, :nb],
                        b2_ap(mi, ms),
                        0.0,
                        op0=Add,
                        op1=Max,
                    )
                    hf_t.append(ht)
                # ---- P11: BM44 K3 + rank-1 bias (closes the psbm group) ----
                for s in range(4):
                    o = ptbm[:, 4 * F1R + s * F2R : 4 * F1R + (s + 1) * F2R]
                    c0 = s * P
                    nc.tensor.matmul(
                        o,
                        h1_t[3][:F1R, c0 : c0 + P],
                        w2_t[3][:F1R, 2 * P : H2],
                        start=False,
                        stop=False,
                        skip_group_check=True,
                    )
                    nc.tensor.matmul(
                        o,
                        ones_t[:, c0 : c0 + P],
                        brow_t[:, F1R : F1R + F2R],
                        start=False,
                        stop=(s == 3),
                        skip_group_check=True,
                    )
                bm2_sb = bmpool.tile([P, 4, F2R], F16, tag="bm2")
                nc.vector.tensor_scalar(
                    bm2_sb[:, :, :], ptbm[:, 4 * F1R :], 0.0, None, op0=Max
                )
                # hf_t[2] is filled by next chunk's P3 transpose+eviction
                hf44 = hfpool.tile([F2R, 512], F16, tag="hf44")
                hf_t.append(hf44)

            return {
                "hf_t": hf_t,
                "b0": b0,
                "nb": nb,
                "ns": ns,
                "bm2_sb": bm2_sb,
                "ptt": ptt,
            }

        def emit_l3(st):
            hf_t, b0, nb, ns = st["hf_t"], st["b0"], st["nb"], st["ns"]
            pt3 = ps3.tile([P, 512], F32, tag="ps3")
            n_mm = ns * (NK3 + 1)
            i = 0
            for s in range(ns):
                c0, c1 = s * P, min((s + 1) * P, nb)
                o = pt3[: c1 - c0, s * A : s * A + A]
                for ki in range(NK3):
                    ks = K3[ki][1]
                    nc.tensor.matmul(
                        o,
                        hf_t[ki][:ks, c0:c1],
                        w3_t[ki][:ks, :A],
                        start=(i == 0),
                        stop=(i == n_mm - 1),
                        skip_group_check=True,
                    )
                    i += 1
                nc.tensor.matmul(
                    o,
                    ones_t[:, c0:c1],
                    brow_t[:, F1R + F2R :],
                    start=False,
                    stop=(i == n_mm - 1),
                    skip_group_check=True,
                )
                i += 1
            ot = opool.tile([P, 4, A], F32, tag="o")
            nc.scalar.activation(ot[:, :ns, :], pt3[:, : ns * A], Tanh)
            s0 = b0 // P
            nc.scalar.dma_start(out=out[:, s0 : s0 + ns, :], in_=ot[:, :ns, :])

        def emit_all():
            prev = None
            for ci, (b0, nb) in enumerate(bchunks):
                prev = emit_chunk(ci, b0, nb, prev)
            # flush: transpose the last bm chunk's BM44 if any, then L3
            if prev["ptt"] is not None:
                ptt = prev["ptt"]
                for s in range(4):
                    nc.tensor.transpose(
                        ptt[:F2R, 512 + s * P : 512 + (s + 1) * P],
                        prev["bm2_sb"][:, s, :],
                        ident_t[:, :],
                    )
                # ACT as copy engine: values are post-relu (>=0) so Relu
                # is the identity; GPSIMD cannot touch PSUM per the verifier
                nc.scalar.activation(
                    prev["hf_t"][2][:F2R, :], ptt[:F2R, 512:1024], Relu
                )
            emit_l3(prev)

        if reps > 1:
            with tc.For_i(0, reps, 1):
                emit_all()
        else:
            emit_all()
    return _legalize_wait_counts(nc) if legalize else nc


def _get_nc(BM):
    if BM not in _nc_cache:
        _nc_cache[BM] = _build(BM)
    return _nc_cache[BM]


def pack_k(mat, nk):
    # [K, N] -> zero-pad K to nk*128 -> [128, nk, N] with row j*128+p of the
    # original at [p, j, :] (zero rows contribute nothing to the contraction)
    kk, nn = mat.shape
    pad = np.zeros((nk * P, nn), np.float16)
    pad[:kk] = mat.astype(np.float16)
    return np.ascontiguousarray(pad.reshape(nk, P, nn).transpose(1, 0, 2))


def pack_bias(b1g, b2s):
    pk = np.zeros((P, BIAS_COLS), np.float32)
    for mi, (m0, ms) in enumerate(M1):
        pk[:ms, mi] = b1g[m0 : m0 + ms]
    for mi, (m0, ms) in enumerate(M2):
        pk[:ms, len(M1) + mi] = b2s[m0 : m0 + ms]
    return pk


def kernel(state, idx, W1, b1, W2, b2, W3, b3):
    global last_run
    state = np.asarray(state, dtype=np.float32)
    idx = np.asarray(idx)
    W1 = np.asarray(W1, dtype=np.float32)
    b1 = np.asarray(b1, dtype=np.float32)
    W2 = np.asarray(W2, dtype=np.float32)
    b2 = np.asarray(b2, dtype=np.float32)
    W3 = np.asarray(W3, dtype=np.float32)
    b3 = np.asarray(b3, dtype=np.float32)
    B = state.shape[0]

    # Host-side routing: idx is sorted in the reference workload; fall back to
    # a stable argsort if not, so grouping stays correct for any input.
    idx_i = idx.astype(np.int64)
    perm = None
    if np.any(np.diff(idx_i) < 0):
        perm = np.argsort(idx_i, kind="stable")
        idx_i = idx_i[perm]
        state = state[perm]
    assert idx_i.min() >= 0 and idx_i.max() < G, "idx out of range [0, G)"
    counts = np.bincount(idx_i, minlength=G)[:G]
    offs = np.concatenate([[0], np.cumsum(counts)])

    BM = max(512, int(-(-counts.max() // P) * P))  # round up to 128 rows
    nc = _get_nc(BM)
    NS = BM // P

    w2p = pack_k(W2, NK2)
    identity = np.eye(P, dtype=np.float16)

    in_maps = []
    for g in range(G):
        seg = state[offs[g] : offs[g + 1]]
        xg = np.zeros((D, BM), np.float32)
        xg[:, : seg.shape[0]] = seg.T
        browg = np.concatenate([b1[g][3 * P :], b2[2 * P :], b3[g]])
        in_maps.append(
            {
                "xP": pack_k(xg, NK1),
                "w1": pack_k(W1[g], NK1),
                "w2": w2p,
                "w3": pack_k(W3[g], NK3),
                "brow": browg.astype(np.float16).reshape(1, -1),
                "ident": identity,
                "bias": pack_bias(b1[g], b2),
            }
        )

    globals()["_last_in_maps"] = in_maps
    try:
        last_run = run_bass_kernel_spmd(nc, in_maps, list(range(NCORES)))
    except ModuleNotFoundError:
        # BASS_TRACE set in an env without the axon NTFF hook: retry untraced
        import os

        os.environ["BASS_NEVER_TRACE"] = "1"
        last_run = run_bass_kernel_spmd(nc, in_maps, list(range(NCORES)))

    out = np.empty((B, A), np.float32)
    for g in range(G):
        og = np.asarray(last_run.results[g]["out"])  # [P, NS, A]
        rows = og.transpose(1, 0, 2).reshape(NS * P, A)
        out[offs[g] : offs[g + 1]] = rows[: counts[g]]
    if perm is not None:
        inv = np.empty_like(perm)
        inv[perm] = np.arange(B)
        out = out[inv]
    return out


# revision 12
# speedup vs baseline: 1.2138x; 1.0386x over previous
"""Trainium2 Bass kernel for the multi-task ActorNetwork (moe_routing).

Architecture (reference): per-sample expert routing over G=8 tasks:
    h1 = relu(x @ W1[idx] + b1[idx])     x:[B,376]  W1:[8,376,400]
    hf = relu(h1 @ W2 + b2)              W2:[400,300]
    a  = tanh(hf @ W3[idx] + b3[idx])    W3:[8,300,17]

Strategy: idx is sorted, and G == n_cores == 8, so we route on the HOST:
core g receives exactly the contiguous rows with idx == g (zero-padded to a
common BM), plus only ITS expert weights. Each core then runs a dense 3-layer
MLP -- no device-side routing, no collectives, and none of the 8x dense
compute the reference does.

Numerics: fp16 operands with fp32 PSUM accumulation; measured end-to-end
max-abs error vs the fp32 reference ~5e-3 on unit-scale outputs.

Matmul cost on the PE is (output free size) x (cycles/row), so the layout is
chosen to minimize streamed output elements:
  * L1/L2 main tiles (128-feature groups) run feature-major: the contraction
    dim sits on SBUF partitions and the 512-sample chunk streams as the
    moving dim.
  * The ragged remainders (h1's last 16 features, hf's last 44) run
    BATCH-major: out[b_slice, F] = x_sliceT.T @ W[:, rem] streams only F
    elements per pass; one PE transpose per 128-sample slice (through fp16
    PSUM) restores the feature-major layout the next layer needs.  The
    current chunk's 16-col block and the PREVIOUS chunk's 44-col block are
    relu'd into one [128, 4, 109] SBUF tile and share the same transpose.
  * L3 is fully batch-major (17 outputs): lhsT = a 128-column slice of hfT,
    rhs = W3 -- ~17 cycles per slice-pass instead of 512 per K-pass.  All L3
    matmuls of a chunk form ONE PSUM accumulation group in one bank writing
    disjoint 17-column slices.
  * ALL biases ride spare contraction rows: x carries a ones-row at 376 and
    W1's K-slab carries b1 there; the transposed remainders carry a ones-row
    (memset column of the [128,4,62] tile) matched by b2/b3 rows in the
    augmented W2/W3 K-slabs.  No bias operands, no rank-1 bias matmuls.

The final (partial) chunk uses the plain feature-major path with M-outer
emission: it keeps the drain chain short and avoids PSUM WAR deadlocks with
3 live L2 groups on 2 banks.

Engine split: PE matmuls+transposes; ACT: L1 relu x2, BM16 relu, hf44T copy,
L3 tanh, out DMA; DVE: L1 relu x1, L2 relu x2, BM44 relu, h1remT copy; Pool:
weight DMAs; SP: x-chunk streaming.  A dummy activation at t~0.5us preloads
the ACT function table off the critical path.
"""

import sys

if "/opt/trn_rl_repo" not in sys.path:
    sys.path.insert(0, "/opt/trn_rl_repo")

from contextlib import ExitStack

import numpy as np

import concourse.bass as bass
import concourse.mybir as mybir
from concourse.bass_utils import run_bass_kernel_spmd
from concourse.tile import TileContext

D, G, H1, H2, A = 376, 8, 400, 300, 17
P = 128
NCORES = 8
F16 = mybir.dt.float16
F32 = mybir.dt.float32

F1R = H1 - 3 * P  # 16: L1 feature remainder
F2R = H2 - 2 * P  # 44: L2 feature remainder
HF44_OFF = 64  # hf44 block column offset (45-partition reads must start at 0/64)
BMC = HF44_OFF + F2R + 1  # 109: combined-transpose tile columns

NK1, NK2, NK3 = 3, 4, 3
KS1 = [128, 128, D - 256]  # x contraction slabs (120-slab holds the ones row)
KS2 = [128, 128, 128, F1R + 1]  # L2 slabs; last = h1rem + ones(b2) row
KS3 = [128, 128, F2R + 1]  # L3 slabs; last = hf44 + ones(b3) row
M1 = [(0, 128), (128, 128), (256, 128), (384, F1R)]
M2 = [(0, 128), (128, 128), (256, F2R)]

_nc_cache = {}
last_run = None  # BassKernelResults of the most recent launch (for profiling)
_last_in_maps = None  # per-core input dicts of the most recent launch

_nop_counter = [0]


def _chunks(total, step):
    return [(o, min(step, total - o)) for o in range(0, total, step)]


def _legalize_wait_counts(nc):
    """This container's walrus encodes at most ONE sync-wait per instruction
    (DMA pseudo-instructions especially). Tile freely emits several. Sequencers
    are in-order, so hoisting the surplus waits onto same-engine NoOps placed
    immediately before the instruction is semantics-preserving."""
    for fn in nc.m.functions:
        for bb in fn.blocks:
            insts = list(bb.instructions)
            out = []
            changed = False
            for inst in insts:
                si = inst.sync_info
                waits = list(si.on_wait) if si is not None and si.on_wait else []
                if len(waits) > 1:
                    changed = True
                    for w in waits[:-1]:
                        _nop_counter[0] += 1
                        nop = mybir.InstNoOp(
                            name=f"waitsplit_nop_{_nop_counter[0]}",
                            engine=inst.engine,
                            ins=[],
                            outs=[],
                            sync_info=mybir.SyncInfo(on_wait=[w], on_update=[]),
                        )
                        out.append(nop)
                    si.on_wait = waits[-1:]
                out.append(inst)
            if changed:
                bb.instructions = out
    return nc


def _build(BM, legalize=True, reps=1):
    """Bass program for one core: dense 3-layer MLP over BM rows.

    reps>1 wraps the body in a hardware For_i loop (benchmarking only)."""
    bchunks = _chunks(BM, 512)
    NS = BM // P  # 128-row output slices

    nc = bass.Bass()
    xP = nc.declare_dram_parameter("xP", [P, NK1, BM], F16, isOutput=False)
    w1 = nc.declare_dram_parameter("w1", [P, NK1, H1], F16, isOutput=False)
    w2 = nc.declare_dram_parameter("w2", [P, NK2, H2], F16, isOutput=False)
    w3 = nc.declare_dram_parameter("w3", [P, NK3, A], F16, isOutput=False)
    ident = nc.declare_dram_parameter("ident", [P, P], F16, isOutput=False)
    # out[p, s, a] = action a of sample s*128 + p (host re-interleaves)
    out = nc.declare_dram_parameter("out", [P, NS, A], F32, isOutput=True)

    Relu = mybir.ActivationFunctionType.Relu
    Tanh = mybir.ActivationFunctionType.Tanh
    Add = mybir.AluOpType.add
    Max = mybir.AluOpType.max

    with TileContext(nc) as tc, ExitStack() as ctx:
        wpool = ctx.enter_context(tc.tile_pool(name="w", bufs=1))
        xpool = ctx.enter_context(tc.tile_pool(name="x", bufs=3))
        h1pool = ctx.enter_context(tc.tile_pool(name="h1", bufs=3))
        hfpool = ctx.enter_context(tc.tile_pool(name="hf", bufs=3))
        bmpool = ctx.enter_context(tc.tile_pool(name="bm", bufs=2))
        opool = ctx.enter_context(tc.tile_pool(name="o", bufs=3))
        ps1 = ctx.enter_context(tc.tile_pool(name="ps1", bufs=3, space="PSUM"))
        ps2 = ctx.enter_context(tc.tile_pool(name="ps2", bufs=2, space="PSUM"))
        psbm = ctx.enter_context(tc.tile_pool(name="psbm", bufs=1, space="PSUM"))
        pst = ctx.enter_context(tc.tile_pool(name="pst", bufs=1, space="PSUM"))
        ps3 = ctx.enter_context(tc.tile_pool(name="ps3", bufs=1, space="PSUM"))

        def load_weights(param, nk, ncols, name, eng):
            tiles = []
            for ki in range(nk):
                t = wpool.tile([P, ncols], F16, tag=f"{name}_{ki}")
                eng.dma_start(out=t[:, :], in_=param[:, ki, :])
                tiles.append(t)
            return tiles

        # Pool ring order is latency-tuned: w1 gates the first L1 passes,
        # ident gates chunk-0's transposes, w3 is needed a full chunk later.
        w1_t = load_weights(w1, NK1, H1, "w1", nc.gpsimd)
        ident_t = wpool.tile([P, P], F16, tag="ident")
        nc.gpsimd.dma_start(out=ident_t[:, :], in_=ident[:, :])
        w3_t = load_weights(w3, NK3, A, "w3", nc.gpsimd)

        # preload the ACT function table off the critical path (the first
        # activation otherwise pays ~1.4us mid-stream)
        seed_t = wpool.tile([1, 1], F32, tag="seed")
        nc.vector.memset(seed_t[:, :], 0.0)
        actw_t = wpool.tile([1, 1], F32, tag="actw")
        nc.scalar.activation(actw_t[:, :], seed_t[:, :], Relu)

        w2_t = load_weights(w2, NK2, H2, "w2", nc.scalar)

        def emit_chunk(ci, b0, nb, prev):
            last = ci == len(bchunks) - 1
            bm = nb == 512 and not last
            ns = (nb + P - 1) // P

            # ---- x DMA (chunk 0: per-K-slab so the first passes start early)
            xt = xpool.tile([P, NK1, 512], F16, tag="x")
            if ci == 0:
                for ki in range(NK1):
                    nc.sync.dma_start(out=xt[:, ki, :nb], in_=xP[:, ki, b0 : b0 + nb])
            else:
                nc.sync.dma_start(out=xt[:, :, :nb], in_=xP[:, :, b0 : b0 + nb])

            # ---- P1: L1 feature-major tiles (b1 rides x's ones row) ----
            nfm1 = 3 if bm else len(M1)
            pts1 = [
                ps1.tile([P, 512], F32, tag="ps1", name=f"ps1_{ci}_{i}")
                for i in range(min(nfm1, 3))
            ]
            if not bm and nfm1 == 4:
                # last/partial chunk: M-outer, M3 reuses M0's bank after its
                # eviction (sequential groups -- no WAR deadlock)
                pts1.append(ps1.tile([P, 512], F32, tag="ps1", name=f"ps1_{ci}_3"))
            h1_t = [None] * NK2
            if ci == 0 and bm:
                order1 = [(ki, mi) for ki in range(NK1) for mi in range(nfm1)]
            else:
                order1 = [(ki, mi) for mi in range(nfm1) for ki in range(NK1)]
            for ki, mi in order1:
                m0, ms = M1[mi]
                nc.tensor.matmul(
                    pts1[mi][:ms, :nb],
                    w1_t[ki][:, m0 : m0 + ms],
                    xt[:, ki, :nb],
                    start=(ki == 0),
                    stop=(ki == NK1 - 1),
                )
                if ki == NK1 - 1:
                    if mi == 3:
                        # FM remainder (partial chunk): augmented ones row
                        # carries b2 into the next layer's contraction
                        ht = h1pool.tile([F1R + 1, nb], F16, tag="h1_3")
                        # engine ops must start at partition 0: fill the whole
                        # tile with ones, the eviction overwrites rows 0:16
                        nc.vector.memset(ht[: F1R + 1, :nb], 1.0)
                        nc.vector.tensor_scalar(
                            ht[:ms, :nb], pts1[mi][:ms, :nb], 0.0, None, op0=Max
                        )
                    else:
                        ht = h1pool.tile([ms, nb], F16, tag=f"h1_{mi}")
                        if mi < 2:
                            nc.scalar.activation(ht[:ms, :nb], pts1[mi][:ms, :nb], Relu)
                        else:
                            nc.vector.tensor_scalar(
                                ht[:ms, :nb], pts1[mi][:ms, :nb], 0.0, None, op0=Max
                            )
                    h1_t[mi] = ht

            # ---- P2: L1 batch-major remainder (16 features) + the combined
            #      transpose staging tile ----
            ptbm = None
            bmc_sb = bmpool.tile([P, 4, BMC], F16, tag="bmc")
            # ones columns (transpose into the b2/b3 contraction rows of
            # h1remT/hf44T); cheap [128,4] writes, re-set each rotation so
            # CoreSim's fresh-tile NaN canaries never leak into the transpose
            nc.vector.memset(bmc_sb[:, :, F1R : F1R + 1], 1.0)
            nc.vector.memset(bmc_sb[:, :, BMC - 1 : BMC], 1.0)
            if bm:
                # psbm bank: cols [0,4*F1R) = BM16, [4*F1R,..) = BM44; ONE
                # accumulation group from the first BM16 mm to the last BM44
                # mm (each slice's first write lands on pending-zero bytes).
                ptbm = psbm.tile([P, 4 * (F1R + F2R)], F32, tag="psbm")
                for s in range(4):
                    o = ptbm[:, s * F1R : (s + 1) * F1R]
                    c0 = s * P
                    for ki in range(NK1):
                        nc.tensor.matmul(
                            o,
                            xt[:, ki, c0 : c0 + P],
                            w1_t[ki][:, 3 * P : H1],
                            start=(s == 0 and ki == 0),
                            stop=False,
                            skip_group_check=True,
                        )
                bm1v = bmc_sb[:, :, :F1R]
                nc.scalar.activation(bm1v, ptbm[:, : 4 * F1R], Relu)

            # ---- P2.5: previous chunk's BM44 relu into the combined tile --
            if prev is not None and prev["ptbm"] is not None:
                nc.vector.tensor_scalar(
                    bmc_sb[:, :, HF44_OFF : HF44_OFF + F2R],
                    prev["ptbm"][:, 4 * F1R :],
                    0.0,
                    None,
                    op0=Max,
                )

            # ---- P4..P7: L2 feature-major K-rounds 0..2 (+T after K1) ----
            nfm2 = 2 if bm else len(M2)
            m2fm = M2[:nfm2]
            pts2 = [
                ps2.tile([P, 512], F32, tag="ps2", name=f"ps2_{ci}_{i}")
                for i in range(min(nfm2, 2))
            ]

            def l2_round(ki):
                ks = KS2[ki]
                for mi, (m0, ms) in enumerate(m2fm):
                    nc.tensor.matmul(
                        pts2[mi][:ms, :nb],
                        w2_t[ki][:ks, m0 : m0 + ms],
                        h1_t[ki][:ks, :nb],
                        start=(ki == 0),
                        stop=(ki == NK2 - 1),
                    )

            def emit_transposes():
                # one [128,109] transpose per slice: rows 0:17 become the
                # augmented h1remT (this chunk), rows 64:109 the augmented
                # hf44T (previous chunk; 45-partition engine reads must start
                # at partition 0 or 64, hence the column gap)
                ptt = pst.tile([BMC, 512], F16, tag="pst")
                for s in range(4):
                    nc.tensor.transpose(
                        ptt[:BMC, s * P : (s + 1) * P],
                        bmc_sb[:, s, :],
                        ident_t[:, :],
                    )
                if prev is not None and prev["ptbm"] is not None:
                    # ACT as copy engine: values are post-relu/ones (>=0)
                    nc.scalar.activation(
                        prev["hf_t"][2][: F2R + 1, :],
                        ptt[HF44_OFF : HF44_OFF + F2R + 1, :],
                        Relu,
                    )
                if bm:
                    h1r = h1pool.tile([F1R + 1, 512], F16, tag="h1r")
                    nc.vector.tensor_scalar(
                        h1r[:, :], ptt[: F1R + 1, :], 0.0, None, op0=Add
                    )
                    h1_t[3] = h1r

            if bm:
                l2_round(0)
                l2_round(1)
                emit_transposes()
                l2_round(2)
            else:
                emit_transposes()
                # M-outer for the drain chunk (see docstring)
                hf_t = []
                for mi, (m0, ms) in enumerate(m2fm):
                    pt = pts2[mi if mi < 2 else 0]
                    for ki in range(NK2):
                        ks = KS2[ki]
                        nc.tensor.matmul(
                            pt[:ms, :nb],
                            w2_t[ki][:ks, m0 : m0 + ms],
                            h1_t[ki][:ks, :nb],
                            start=(ki == 0),
                            stop=(ki == NK2 - 1),
                        )
                    if mi == 2:
                        ht = hfpool.tile([F2R + 1, nb], F16, tag="hf_2")
                        nc.vector.memset(ht[: F2R + 1, :nb], 1.0)
                        nc.vector.tensor_scalar(
                            ht[:ms, :nb], pt[:ms, :nb], 0.0, None, op0=Max
                        )
                    else:
                        ht = hfpool.tile([ms, nb], F16, tag=f"hf_{mi}")
                        if mi == 1:
                            nc.scalar.activation(ht[:ms, :nb], pt[:ms, :nb], Relu)
                        else:
                            nc.vector.tensor_scalar(
                                ht[:ms, :nb], pt[:ms, :nb], 0.0, None, op0=Max
                            )
                    hf_t.append(ht)

            # ---- P8: BM44 K0..K2 ----
            if bm:
                for s in range(4):
                    o = ptbm[:, 4 * F1R + s * F2R : 4 * F1R + (s + 1) * F2R]
                    c0 = s * P
                    for ki in range(3):
                        nc.tensor.matmul(
                            o,
                            h1_t[ki][:, c0 : c0 + P],
                            w2_t[ki][:, 2 * P : H2],
                            start=False,
                            stop=False,
                            skip_group_check=True,
                        )

            # ---- P9: L3 of the previous chunk, tanh, out DMA ----
            if prev is not None:
                emit_l3(prev)

            if bm:
                # ---- P10: L2 K3 round (augmented h1rem: adds b2) ----
                hf_t = []
                ks = KS2[3]
                for mi, (m0, ms) in enumerate(m2fm):
                    nc.tensor.matmul(
                        pts2[mi][:ms, :nb],
                        w2_t[3][:ks, m0 : m0 + ms],
                        h1_t[3][:ks, :nb],
                        start=False,
                        stop=True,
                    )
                    ht = hfpool.tile([ms, nb], F16, tag=f"hf_{mi}")
                    nc.vector.tensor_scalar(
                        ht[:ms, :nb], pts2[mi][:ms, :nb], 0.0, None, op0=Max
                    )
                    hf_t.append(ht)
                # ---- P11: BM44 K3 (closes the psbm group) ----
                for s in range(4):
                    o = ptbm[:, 4 * F1R + s * F2R : 4 * F1R + (s + 1) * F2R]
                    c0 = s * P
                    nc.tensor.matmul(
                        o,
                        h1_t[3][: KS2[3], c0 : c0 + P],
                        w2_t[3][: KS2[3], 2 * P : H2],
                        start=False,
                        stop=(s == 3),
                        skip_group_check=True,
                    )
                # hf_t[2] (augmented hf44T) is filled by the NEXT chunk's
                # combined transpose
                hf44 = hfpool.tile([F2R + 1, 512], F16, tag="hf44")
                hf_t.append(hf44)

            return {
                "hf_t": hf_t,
                "b0": b0,
                "nb": nb,
                "ns": ns,
                "ptbm": ptbm,
            }

        def emit_l3(st):
            hf_t, b0, nb, ns = st["hf_t"], st["b0"], st["nb"], st["ns"]
            pt3 = ps3.tile([P, 512], F32, tag="ps3")
            n_mm = ns * NK3
            i = 0
            for s in range(ns):
                c0, c1 = s * P, min((s + 1) * P, nb)
                o = pt3[: c1 - c0, s * A : s * A + A]
                for ki in range(NK3):
                    ks = KS3[ki]
                    nc.tensor.matmul(
                        o,
                        hf_t[ki][:ks, c0:c1],
                        w3_t[ki][:ks, :A],
                        start=(i == 0),
                        stop=(i == n_mm - 1),
                        skip_group_check=True,
                    )
                    i += 1
            ot = opool.tile([P, 4, A], F32, tag="o")
            nc.scalar.activation(ot[:, :ns, :], pt3[:, : ns * A], Tanh)
            s0 = b0 // P
            nc.scalar.dma_start(out=out[:, s0 : s0 + ns, :], in_=ot[:, :ns, :])

        def emit_all():
            prev = None
            for ci, (b0, nb) in enumerate(bchunks):
                prev = emit_chunk(ci, b0, nb, prev)
            emit_l3(prev)

        if reps > 1:
            with tc.For_i(0, reps, 1):
                emit_all()
        else:
            emit_all()
    return _legalize_wait_counts(nc) if legalize else nc


def _get_nc(BM):
    if BM not in _nc_cache:
        _nc_cache[BM] = _build(BM)
    return _nc_cache[BM]


def pack_k(mat, nk):
    # [K, N] -> zero-pad K to nk*128 -> [128, nk, N] with row j*128+p of the
    # original at [p, j, :] (zero rows contribute nothing to the contraction)
    kk, nn = mat.shape
    pad = np.zeros((nk * P, nn), np.float16)
    pad[:kk] = mat.astype(np.float16)
    return np.ascontiguousarray(pad.reshape(nk, P, nn).transpose(1, 0, 2))


def _aug(mat, row):
    # append a bias row to the contraction dim
    return np.concatenate([mat, row.reshape(1, -1)], axis=0)


def kernel(state, idx, W1, b1, W2, b2, W3, b3):
    global last_run
    state = np.asarray(state, dtype=np.float32)
    idx = np.asarray(idx)
    W1 = np.asarray(W1, dtype=np.float32)
    b1 = np.asarray(b1, dtype=np.float32)
    W2 = np.asarray(W2, dtype=np.float32)
    b2 = np.asarray(b2, dtype=np.float32)
    W3 = np.asarray(W3, dtype=np.float32)
    b3 = np.asarray(b3, dtype=np.float32)
    B = state.shape[0]

    # Host-side routing: idx is sorted in the reference workload; fall back to
    # a stable argsort if not, so grouping stays correct for any input.
    idx_i = idx.astype(np.int64)
    perm = None
    if np.any(np.diff(idx_i) < 0):
        perm = np.argsort(idx_i, kind="stable")
        idx_i = idx_i[perm]
        state = state[perm]
    assert idx_i.min() >= 0 and idx_i.max() < G, "idx out of range [0, G)"
    counts = np.bincount(idx_i, minlength=G)[:G]
    offs = np.concatenate([[0], np.cumsum(counts)])

    BM = max(512, int(-(-counts.max() // P) * P))  # round up to 128 rows
    nc = _get_nc(BM)
    NS = BM // P

    # W2 augmented with the b2 row (the kernel's KS2[-1] = 17 rows cover
    # h1[384:400] + the ones row of h1remT)
    w2p = pack_k(_aug(W2, b2), NK2)
    identity = np.eye(P, dtype=np.float16)

    in_maps = []
    for g in range(G):
        seg = state[offs[g] : offs[g + 1]]
        xg = np.zeros((D + 1, BM), np.float32)
        xg[:D, : seg.shape[0]] = seg.T
        xg[D, :] = 1.0  # ones row -> b1 via W1's augmented row
        in_maps.append(
            {
                "xP": pack_k(xg, NK1),
                "w1": pack_k(_aug(W1[g], b1[g]), NK1),
                "w2": w2p,
                "w3": pack_k(_aug(W3[g], b3[g]), NK3),
                "ident": identity,
            }
        )

    globals()["_last_in_maps"] = in_maps
    try:
        last_run = run_bass_kernel_spmd(nc, in_maps, list(range(NCORES)))
    except ModuleNotFoundError:
        # BASS_TRACE set in an env without the axon NTFF hook: retry untraced
        import os

        os.environ["BASS_NEVER_TRACE"] = "1"
        last_run = run_bass_kernel_spmd(nc, in_maps, list(range(NCORES)))

    out = np.empty((B, A), np.float32)
    for g in range(G):
        og = np.asarray(last_run.results[g]["out"])  # [P, NS, A]
        rows = og.transpose(1, 0, 2).reshape(NS * P, A)
        out[offs[g] : offs[g + 1]] = rows[: counts[g]]
    if perm is not None:
        inv = np.empty_like(perm)
        inv[perm] = np.arange(B)
        out = out[inv]
    return out
